# revision 28
# baseline (speedup 1.0000x reference)
"""Trainium2 Bass kernel for nn_ClusteringLayer (vq_codebook soft assignments).

Computes q[n, k] = r / sum_k r with r = 1 / (1 + |x_n - c_k|^2), data-parallel
over 8 NeuronCores (x sharded on the sample axis, centroids replicated).

Design (v8):
  * Exact expansion u = (1 + |x_n|^2) + |c_k|^2 - 2 x_n.c_k.  The cross term
    is ONE fp8e4 DoubleRow matmul per 128-sample tile (host packs (32*x)^T
    DoubleRow [f_lo, (i, m)]; cw8 = fp8(-16 c^T) packed [f_lo, (i, k)]; the
    256x product scale is undone by the ACT scale immediate).
  * The affine part 256*(A_n + csq_k) rides one K=128 bf16 matmul per tile
    (zero-padded from 18 rows ON THE HOST -- tiny-K matmuls run ~3x slower
    and break PE pipelining): lhsT rows = [a_hi[t'] x8, a_lo[t'] x8, 1, 1]
    sliced per group from a static [128, 4096] table; rhs rows =
    [delta_t' x8, delta_t' x8, 256csq_hi, 256csq_lo] static [128, 2048].
    Each affine matmul must target EXACTLY the open accumulation region of
    its cross matmul (start=True ... stop=True per 256-col tile region);
    batching FD=512 across two regions, or reordering after the next
    cross's start=True in the same PSUM bank, silently drops the data.
  * Samples interleaved n = g*1024 + p*8 + t so each output-DMA partition
    line is one contiguous 4 KiB burst in original sample order.
  * One WIDE ACT Reciprocal per 8-tile group ([128, 2048] f32 PSUM -> bf16
    SBUF) amortizes ACT's ~300 ns/inst overhead; the extra /256 in the
    scale keeps the LUT input in the ~[0.8, 4.3] domain where the (bass-
    gated) Reciprocal LUT measures ~1e-5 rel err; the constant factor on r
    cancels in the row normalization.
  * Row sums: bf16 halving-add chain at 2x (128->64->32) then a short 1x
    tensor_reduce (tensor_reduce has no 16-bit uop).  sinv = 1/sums rides
    the ACT queue (patched Reciprocal) to keep DVE lean.
  * q-muls: per-tile tensor_scalar_mul, 6 tiles on DVE (2x; the f32 [P,1]
    scalar AP occupies a read port, and scalar APs are f32-only, so the 4x
    all-16-bit mode is unreachable) + 2 tiles on ACT (Copy with
    per-partition scale).  Software-pipelined one group back so the ACT
    FIFO never holds recip(g+1) hostage to group g's DVE sums chain.
  * GpSimd does ONLY output-DMA triggers: its tensor ops are ~20x slower
    than DVE and contend for the SBUF port shared with DVE (measured 2-3x
    stretch of every DVE op).  The DMA compute engine only supports
    add-class accum ops (no mult), and strided SBUF->SBUF accum DMAs
    shatter into 256 B descriptors that flood all 16 queues.
"""

from contextlib import ExitStack

import numpy as np

import concourse.bacc as bacc
import concourse.bass as bass
import concourse.tile as tile
from concourse import mybir
from concourse.bass_utils import run_bass_kernel_spmd

N_CORES = 8
N_SAMPLES = 262144
N_FEAT = 256
N_CLUST = 256
S = N_SAMPLES // N_CORES  # samples per core
P = 128  # partitions / samples per tile
T_GROUP = 8  # tiles per PSUM group (4 banks)
NW = P * T_GROUP  # 1024 samples per group
SUPER = 2  # groups per input-DMA superblock
G = S // NW  # 32 groups per core
SW = NW * SUPER  # samples per superblock
KA = 18  # affine matmul contraction: 8 a_hi + 8 a_lo + csq_hi + csq_lo

BF16 = mybir.dt.bfloat16
F32 = mybir.dt.float32
FP8 = mybir.dt.float8e4
NP_BF16 = mybir.dt.np(BF16)
NP_FP8 = mybir.dt.np(FP8)

XS_X = 32.0  # fp8 centering for x
XS_C = 8.0  # fp8 centering for -2c
XSCALE = XS_X * XS_C  # product scale undone by the ACT scale immediate

# Set by test harness to capture an NTFF profile; kernel output is unaffected.
RUN_TRACE = False
LAST_RESULT = None


def _trim_tile_tail():
    if getattr(tile.TileContext, "_tail_trimmed", False):
        return
    from concourse.vector_clock import ScopedClock

    def _drain_and_barrier(self, tick_clock, wait_clock):
        nc = self.nc
        drain_inst = nc.sync.drain()
        wait_clock.add_sem_waits(
            drain_inst.ins, ScopedClock({None: tick_clock.global_clock})
        )
        nc.all_engine_barrier()
        popped = nc._tile_sem_poison_stack.pop()
        assert popped is self._sem_poison
        # skip clear_and_free_semaphores + second barrier: the kernel preamble
        # clears all sems, so end-of-kernel clears only stretch the tail.
        self.sems.allocated()

    tile.TileContext._drain_and_barrier = _drain_and_barrier
    tile.TileContext._tail_trimmed = True


def _build_nc() -> bacc.Bacc:
    _trim_tile_tail()
    nc = bacc.Bacc()
    # DoubleRow-packed fp8 lhsT: xdr[f_lo, (T, i, m)] = fp8(32*x)[n(T,m), i*128+f_lo]
    xdr = nc.declare_dram_parameter("xdr", [P, 2 * S], FP8, isOutput=False)
    # DoubleRow-packed fp8 rhs: cw8[f_lo, (i, k)] = fp8(-16*c^T)[i*128+f_lo, k]
    cw8 = nc.declare_dram_parameter("cw8", [P, 2 * N_CLUST], FP8, isOutput=False)
    # Affine lhsT table: art[j, g*128+p]; rows 0..7 = 256*A_n hi (t'=j),
    # rows 8..15 = lo, rows 16..17 = 1.0.  A_n = 1 + |x_n|^2.
    art = nc.declare_dram_parameter("art", [P, G * P], BF16, isOutput=False)
    # Affine rhs pattern: pat[j, t*256+k]; rows 0..7 = delta(t==j), rows
    # 8..15 = delta(t==j-8), row 16/17 = 256*csq hi/lo tiled 8x.
    pat = nc.declare_dram_parameter("pat", [P, T_GROUP * N_CLUST], BF16, isOutput=False)
    q = nc.declare_dram_parameter("q", [S, N_CLUST], BF16, isOutput=True)

    # sample n = (g*128 + p)*8 + t  ->  stage[p, t*256 + k] of group g:
    # each partition's 2048 bf16 (4 KiB) are one contiguous DRAM burst.
    qv = q.rearrange("(g p t) k -> g p (t k)", p=P, t=T_GROUP)

    with tile.TileContext(nc) as tc, ExitStack() as ctx:
        statics = ctx.enter_context(tc.tile_pool(name="statics", bufs=1))
        xpool = ctx.enter_context(tc.tile_pool(name="x", bufs=3))
        rpool = ctx.enter_context(tc.tile_pool(name="r", bufs=6))
        spool = ctx.enter_context(tc.tile_pool(name="small", bufs=8))
        opool = ctx.enter_context(tc.tile_pool(name="out", bufs=5))
        pspool = ctx.enter_context(tc.tile_pool(name="ps", bufs=2, space="PSUM"))

        # Dummy 1-elem Reciprocal so walrus's ACT_TABLE_LOAD (~2.7us) runs
        # during the initial input DMA instead of before the first real recip.
        warm = statics.tile([P, 2], F32, tag="warm")
        nc.vector.memset(warm, 1.0)
        inst = nc.scalar.activation(
            out=warm[:, 0:1], in_=warm[:, 1:2], bias=1.0,
            func=mybir.ActivationFunctionType.Copy,
        )
        inst.ins.func = mybir.ActivationFunctionType.Reciprocal

        # art/pat zero-padded to K=128 on the HOST: tiny-K matmuls
        # (tile_size 32) run ~3x slower on HW and break PE pipelining with
        # the DR crosses; host padding avoids startup memsets.  The DMAs
        # are split so group 0's slices land first.
        cw8_s = statics.tile([P, 2 * N_CLUST], FP8)
        nc.sync.dma_start(out=cw8_s, in_=cw8[:, :])
        art_s = statics.tile([P, G * P], BF16)
        pat_s = statics.tile([P, T_GROUP * N_CLUST], BF16)
        nc.sync.dma_start(out=pat_s[:, 0:N_CLUST], in_=pat[:, 0:N_CLUST])
        nc.sync.dma_start(out=art_s[:, 0:P], in_=art[:, 0:P])
        cw8_dr = cw8_s.rearrange("p (i n) -> p i n", i=2)

        def _emit_muls(st):
            # q = r * sinv: per-tile tensor_scalar_mul (2x on DVE; the f32
            # scalar AP occupies a read port so 4x is out of reach), 6
            # tiles on DVE + 2 on ACT (Copy with per-partition scale) to
            # balance the two engines.  GpSimd compute is off-limits: its
            # ops are ~20x slower AND contend for the SBUF port shared
            # with DVE, stretching every DVE op 2-3x.  The DMA compute
            # engine only supports add-class accum ops, not mult.
            r_, sinv_, gi_ = st
            stage = opool.tile([P, T_GROUP * N_CLUST], BF16)
            for t in range(T_GROUP):
                ksl = slice(t * N_CLUST, (t + 1) * N_CLUST)
                if t < 6:
                    nc.vector.tensor_scalar_mul(
                        out=stage[:, ksl], in0=r_[:, ksl],
                        scalar1=sinv_[:, t : t + 1],
                    )
                elif t == 6:
                    # split tile 6 between the engines: the optimal
                    # DVE/ACT balance point is fractional.
                    hsl = slice(t * N_CLUST, t * N_CLUST + N_CLUST // 2)
                    nc.vector.tensor_scalar_mul(
                        out=stage[:, hsl], in0=r_[:, hsl],
                        scalar1=sinv_[:, t : t + 1],
                    )
                    hsl = slice(t * N_CLUST + N_CLUST // 2, (t + 1) * N_CLUST)
                    nc.scalar.activation(
                        out=stage[:, hsl], in_=r_[:, hsl],
                        func=mybir.ActivationFunctionType.Copy,
                        bias=0.0, scale=sinv_[:, t : t + 1],
                    )
                else:
                    nc.scalar.activation(
                        out=stage[:, ksl], in_=r_[:, ksl],
                        func=mybir.ActivationFunctionType.Copy,
                        bias=0.0, scale=sinv_[:, t : t + 1],
                    )
            nc.gpsimd.dma_start(out=qv[gi_], in_=stage)

        prev = None
        for sb in range(G // SUPER):
            s0 = sb * SW
            xs = xpool.tile([P, 2 * SW], FP8, tag="xs")
            if sb == 0:
                # halve the first loads so group 0's matmuls start sooner
                for hh in range(2):
                    hsl = slice(hh * SW, (hh + 1) * SW)
                    nc.sync.dma_start(out=xs[:, hsl], in_=xdr[:, 2 * s0 + hh * SW : 2 * s0 + (hh + 1) * SW])
                # bulk affine tables land after the first x tiles so the
                # first cross matmuls are not stuck behind 1.5 MB of DMA
                nc.sync.dma_start(out=pat_s[:, N_CLUST:], in_=pat[:, N_CLUST:])
                nc.sync.dma_start(out=art_s[:, P : 2 * P], in_=art[:, P : 2 * P])
                nc.sync.dma_start(out=art_s[:, 2 * P :], in_=art[:, 2 * P :])
            else:
                nc.sync.dma_start(out=xs, in_=xdr[:, 2 * s0 : 2 * (s0 + SW)])

            for gl in range(SUPER):
                gi = sb * SUPER + gl
                ps = pspool.tile([P, T_GROUP * N_CLUST], F32)
                for t in range(T_GROUP):
                    tsl = slice(t * N_CLUST, (t + 1) * N_CLUST)
                    xcol = (gl * T_GROUP + t) * 2 * P
                    nc.tensor.matmul(
                        ps[:, tsl],
                        lhsT=xs[:, xcol : xcol + 2 * P].rearrange(
                            "p (i m) -> p i m", i=2
                        ),
                        rhs=cw8_dr,
                        start=True, stop=False,
                        perf_mode=mybir.MatmulPerfMode.DoubleRow,
                    )
                    # affine part: ps[p, (t,k)] += 256*(A_n + csq_k): K=128
                    # (zero-padded from 18) bf16 matmul on the exact same
                    # PSUM region (accumulation groups must match the open
                    # region, so this cannot batch across tiles or reorder
                    # after the next cross).
                    nc.tensor.matmul(
                        ps[:, tsl],
                        lhsT=art_s[:, gi * P : (gi + 1) * P],
                        rhs=pat_s[:, tsl],
                        start=False, stop=True,
                    )
                # r = 256 / (psum/256) = const * 1/u: one wide ACT op per
                # group straight from PSUM.  The extra /256 keeps the LUT
                # input in the ~[0.8, 4.3] domain where Reciprocal is
                # accurate; the constant factor on r cancels in the row
                # normalization.
                r = rpool.tile([P, T_GROUP * N_CLUST], BF16)
                inst = nc.scalar.activation(
                    out=r, in_=ps, bias=0.0, scale=1.0 / (XSCALE * 256.0),
                    func=mybir.ActivationFunctionType.Copy,
                )
                inst.ins.func = mybir.ActivationFunctionType.Reciprocal

                # Software pipeline: emit the PREVIOUS group's q-muls now,
                # after this group's recip.  Otherwise the ACT-side muls of
                # group g sit in the strict ACT FIFO between recip(g) and
                # recip(g+1) while waiting on the DVE sums chain, and the
                # two engines take turns idling.
                if prev is not None:
                    _emit_muls(prev)

                # Row sums: halving-add chain at 2x then a short 1x
                # tensor_reduce (tensor_reduce has no 16-bit uop; the chain
                # minimizes its 1x element count).
                r3 = r.rearrange("p (t k) -> p t k", t=T_GROUP)
                hw = 128
                hsrc = r3
                while hw >= 32:
                    ht = spool.tile([P, T_GROUP * hw], BF16, tag=f"h{hw}")
                    ht3 = ht.rearrange("p (t k) -> p t k", t=T_GROUP)
                    nc.vector.tensor_tensor(
                        out=ht3, in0=hsrc[:, :, 0:hw], in1=hsrc[:, :, hw : 2 * hw],
                        op=mybir.AluOpType.add,
                    )
                    hsrc = ht3
                    hw //= 2
                sums = spool.tile([P, T_GROUP], F32, tag="sums")
                nc.vector.tensor_reduce(
                    out=sums, in_=hsrc,
                    axis=mybir.AxisListType.X,
                    op=mybir.AluOpType.add,
                )
                # sinv on ACT (patched Reciprocal; domain sums~128 is well
                # inside the LUT's accurate range).  Emitted after the
                # previous group's ACT copies, so the only FIFO exposure is
                # recip(g+1) briefly waiting on this group's sums.
                sinv = spool.tile([P, T_GROUP], F32, tag="sinv")
                inst = nc.scalar.activation(
                    out=sinv, in_=sums, bias=0.0, scale=1.0,
                    func=mybir.ActivationFunctionType.Copy,
                )
                inst.ins.func = mybir.ActivationFunctionType.Reciprocal
                prev = (r, sinv, gi)
        _emit_muls(prev)
    nc.finalize()
    return nc


_NC_CACHE = None


def _get_nc():
    global _NC_CACHE
    if _NC_CACHE is None:
        _NC_CACHE = _build_nc()
    return _NC_CACHE


def _hi_lo_bf16(v: np.ndarray) -> tuple[np.ndarray, np.ndarray]:
    hi = v.astype(NP_BF16)
    lo = (v - hi.astype(np.float32)).astype(NP_BF16)
    return hi, lo


def kernel(x: np.ndarray, centroids: np.ndarray) -> np.ndarray:
    global LAST_RESULT
    x = np.ascontiguousarray(np.asarray(x, dtype=np.float32))
    c = np.ascontiguousarray(np.asarray(centroids, dtype=np.float32))
    assert x.shape == (N_SAMPLES, N_FEAT) and c.shape == (N_CLUST, N_FEAT)

    # Shared (replicated) centroid-side operands.
    cw8_flat = (-2.0 * XS_C * c.T).astype(NP_FP8)  # [F, K] fp8
    cw8_host = np.ascontiguousarray(
        cw8_flat.reshape(2, P, N_CLUST).transpose(1, 0, 2).reshape(P, 2 * N_CLUST)
    )
    c_sq = np.einsum("kf,kf->k", c.astype(np.float64), c.astype(np.float64))
    c_sq = (XSCALE * c_sq).astype(np.float32)
    csq_hi, csq_lo = _hi_lo_bf16(c_sq)
    pat_host = np.zeros((P, T_GROUP * N_CLUST), dtype=NP_BF16)
    for t in range(T_GROUP):
        ksl = slice(t * N_CLUST, (t + 1) * N_CLUST)
        pat_host[t, ksl] = 1.0
        pat_host[8 + t, ksl] = 1.0
        pat_host[16, ksl] = csq_hi
        pat_host[17, ksl] = csq_lo

    # m-th column consumed by the kernel (tile-major) is sample n = perm[m],
    # chosen so output partition lines are contiguous 4 KiB bursts in original
    # sample order.
    perm = np.arange(S).reshape(G, P, T_GROUP).transpose(0, 2, 1).reshape(-1)

    in_maps = []
    for i in range(N_CORES):
        xs = x[i * S : (i + 1) * S]  # [S, F]
        x_sq = np.einsum("nf,nf->n", xs.astype(np.float64), xs.astype(np.float64))
        a = (XSCALE * (1.0 + x_sq)).astype(np.float32)  # [S] = 256*A_n
        xs8 = (xs[perm] * XS_X).astype(NP_FP8)  # [S, F] fp8
        # DoubleRow pack: [T, m, i, f_lo] -> [f_lo, T, i, m]
        xdr_host = np.ascontiguousarray(
            xs8.reshape(G * T_GROUP, P, 2, P).transpose(3, 0, 2, 1).reshape(P, 2 * S)
        )
        a_hi, a_lo = _hi_lo_bf16(a)
        # art[t', g*128+p] = a_{hi,lo}[(g*128+p)*8 + t']; rows 16/17 = 1.
        art_host = np.zeros((P, G * P), dtype=NP_BF16)
        art_host[0:8] = a_hi.reshape(G * P, T_GROUP).T
        art_host[8:16] = a_lo.reshape(G * P, T_GROUP).T
        art_host[16:18] = 1.0
        in_maps.append(
            {"xdr": xdr_host, "art": np.ascontiguousarray(art_host),
             "cw8": cw8_host, "pat": pat_host}
        )

    nc = _get_nc()
    res = run_bass_kernel_spmd(
        nc, in_maps, list(range(N_CORES)), trace=RUN_TRACE
    )
    LAST_RESULT = res

    out = np.empty((N_SAMPLES, N_CLUST), dtype=np.float32)
    for i in range(N_CORES):
        out[i * S : (i + 1) * S] = res.results[i]["q"].astype(np.float32)
    return out


# revision 29
# speedup vs baseline: 1.0427x; 1.0427x over previous
"""Trainium2 Bass kernel for nn_ClusteringLayer (vq_codebook soft assignments).

Computes q[n, k] = r / sum_k r with r = 1 / (1 + |x_n - c_k|^2), data-parallel
over 8 NeuronCores (x sharded on the sample axis, centroids replicated).

Design (v8):
  * Exact expansion u = (1 + |x_n|^2) + |c_k|^2 - 2 x_n.c_k.  The cross term
    is ONE fp8e4 DoubleRow matmul per 128-sample tile (host packs (32*x)^T
    DoubleRow [f_lo, (i, m)]; cw8 = fp8(-16 c^T) packed [f_lo, (i, k)]; the
    256x product scale is undone by the ACT scale immediate).
  * The affine part 256*(A_n + csq_k) rides one K=128 bf16 matmul per tile
    (zero-padded from 18 rows ON THE HOST -- tiny-K matmuls run ~3x slower
    and break PE pipelining): lhsT rows = [a_hi[t'] x8, a_lo[t'] x8, 1, 1]
    sliced per group from a static [128, 4096] table; rhs rows =
    [delta_t' x8, delta_t' x8, 256csq_hi, 256csq_lo] static [128, 2048].
    Each affine matmul must target EXACTLY the open accumulation region of
    its cross matmul (start=True ... stop=True per 256-col tile region);
    batching FD=512 across two regions, or reordering after the next
    cross's start=True in the same PSUM bank, silently drops the data.
  * Samples interleaved n = g*1024 + p*8 + t so each output-DMA partition
    line is one contiguous 4 KiB burst in original sample order.
  * One WIDE ACT Reciprocal per 8-tile group ([128, 2048] f32 PSUM -> bf16
    SBUF) amortizes ACT's ~300 ns/inst overhead; the extra /256 in the
    scale keeps the LUT input in the ~[0.8, 4.3] domain where the (bass-
    gated) Reciprocal LUT measures ~1e-5 rel err; the constant factor on r
    cancels in the row normalization.
  * Row sums: bf16 halving-add chain at 2x (128->64->32) then a short 1x
    tensor_reduce (tensor_reduce has no 16-bit uop).  sinv = 1/sums rides
    the ACT queue (patched Reciprocal) to keep DVE lean.
  * q-muls: per-tile tensor_scalar_mul, 6 tiles on DVE (2x; the f32 [P,1]
    scalar AP occupies a read port, and scalar APs are f32-only, so the 4x
    all-16-bit mode is unreachable) + 2 tiles on ACT (Copy with
    per-partition scale).  Software-pipelined one group back so the ACT
    FIFO never holds recip(g+1) hostage to group g's DVE sums chain.
  * GpSimd does ONLY output-DMA triggers: its tensor ops are ~20x slower
    than DVE and contend for the SBUF port shared with DVE (measured 2-3x
    stretch of every DVE op).  The DMA compute engine only supports
    add-class accum ops (no mult), and strided SBUF->SBUF accum DMAs
    shatter into 256 B descriptors that flood all 16 queues.
"""

from contextlib import ExitStack

import numpy as np

import concourse.bacc as bacc
import concourse.bass as bass
import concourse.tile as tile
from concourse import mybir
from concourse.bass_utils import run_bass_kernel_spmd

N_CORES = 8
N_SAMPLES = 262144
N_FEAT = 256
N_CLUST = 256
S = N_SAMPLES // N_CORES  # samples per core
P = 128  # partitions / samples per tile
T_GROUP = 8  # tiles per PSUM group (4 banks)
NW = P * T_GROUP  # 1024 samples per group
SUPER = 2  # groups per input-DMA superblock
G = S // NW  # 32 groups per core
SW = NW * SUPER  # samples per superblock
KA = 18  # affine matmul contraction: 8 a_hi + 8 a_lo + csq_hi + csq_lo

BF16 = mybir.dt.bfloat16
F32 = mybir.dt.float32
FP8 = mybir.dt.float8e4
NP_BF16 = mybir.dt.np(BF16)
NP_FP8 = mybir.dt.np(FP8)

XS_X = 32.0  # fp8 centering for x
XS_C = 8.0  # fp8 centering for -2c
XSCALE = XS_X * XS_C  # product scale undone by the ACT scale immediate

# Set by test harness to capture an NTFF profile; kernel output is unaffected.
RUN_TRACE = False
LAST_RESULT = None


def _trim_tile_tail():
    if getattr(tile.TileContext, "_tail_trimmed", False):
        return
    from concourse.vector_clock import ScopedClock

    def _drain_and_barrier(self, tick_clock, wait_clock):
        nc = self.nc
        drain_inst = nc.sync.drain()
        wait_clock.add_sem_waits(
            drain_inst.ins, ScopedClock({None: tick_clock.global_clock})
        )
        nc.all_engine_barrier()
        popped = nc._tile_sem_poison_stack.pop()
        assert popped is self._sem_poison
        # skip clear_and_free_semaphores + second barrier: the kernel preamble
        # clears all sems, so end-of-kernel clears only stretch the tail.
        self.sems.allocated()

    tile.TileContext._drain_and_barrier = _drain_and_barrier
    tile.TileContext._tail_trimmed = True


def _build_nc() -> bacc.Bacc:
    _trim_tile_tail()
    nc = bacc.Bacc()
    # DoubleRow-packed fp8 lhsT: xdr[f_lo, (T, i, m)] = fp8(32*x)[n(T,m), i*128+f_lo]
    xdr = nc.declare_dram_parameter("xdr", [P, 2 * S], FP8, isOutput=False)
    # DoubleRow-packed fp8 rhs: cw8[f_lo, (i, k)] = fp8(-16*c^T)[i*128+f_lo, k]
    cw8 = nc.declare_dram_parameter("cw8", [P, 2 * N_CLUST], FP8, isOutput=False)
    # Affine lhsT table: art[j, g*128+p]; rows 0..7 = 256*A_n hi (t'=j),
    # rows 8..15 = lo, rows 16..17 = 1.0.  A_n = 1 + |x_n|^2.
    art = nc.declare_dram_parameter("art", [P, G * P], BF16, isOutput=False)
    # Affine rhs pattern: pat[j, t*256+k]; rows 0..7 = delta(t==j), rows
    # 8..15 = delta(t==j-8), row 16/17 = 256*csq hi/lo tiled 8x.
    pat = nc.declare_dram_parameter("pat", [P, T_GROUP * N_CLUST], BF16, isOutput=False)
    q = nc.declare_dram_parameter("q", [S, N_CLUST], BF16, isOutput=True)

    # sample n = (g*128 + p)*8 + t  ->  stage[p, t*256 + k] of group g:
    # each partition's 2048 bf16 (4 KiB) are one contiguous DRAM burst.
    qv = q.rearrange("(g p t) k -> g p (t k)", p=P, t=T_GROUP)

    with tile.TileContext(nc) as tc, ExitStack() as ctx:
        statics = ctx.enter_context(tc.tile_pool(name="statics", bufs=1))
        xpool = ctx.enter_context(tc.tile_pool(name="x", bufs=3))
        rpool = ctx.enter_context(tc.tile_pool(name="r", bufs=6))
        spool = ctx.enter_context(tc.tile_pool(name="small", bufs=8))
        opool = ctx.enter_context(tc.tile_pool(name="out", bufs=5))
        pspool = ctx.enter_context(tc.tile_pool(name="ps", bufs=2, space="PSUM"))

        # Dummy 1-elem Reciprocal so walrus's ACT_TABLE_LOAD (~2.7us) runs
        # during the initial input DMA instead of before the first real recip.
        warm = statics.tile([P, 2], F32, tag="warm")
        nc.vector.memset(warm, 1.0)
        inst = nc.scalar.activation(
            out=warm[:, 0:1], in_=warm[:, 1:2], bias=1.0,
            func=mybir.ActivationFunctionType.Copy,
        )
        inst.ins.func = mybir.ActivationFunctionType.Reciprocal

        # art/pat zero-padded to K=128 on the HOST: tiny-K matmuls
        # (tile_size 32) run ~3x slower on HW and break PE pipelining with
        # the DR crosses; host padding avoids startup memsets.  The DMAs
        # are split so group 0's slices land first.
        cw8_s = statics.tile([P, 2 * N_CLUST], FP8)
        nc.sync.dma_start(out=cw8_s, in_=cw8[:, :])
        art_s = statics.tile([P, G * P], BF16)
        pat_s = statics.tile([P, T_GROUP * N_CLUST], BF16)
        nc.sync.dma_start(out=pat_s[:, 0:N_CLUST], in_=pat[:, 0:N_CLUST])
        nc.sync.dma_start(out=art_s[:, 0:P], in_=art[:, 0:P])
        cw8_dr = cw8_s.rearrange("p (i n) -> p i n", i=2)

        def _emit_muls(st):
            # q = r * sinv: per-tile tensor_scalar_mul (2x on DVE; the f32
            # scalar AP occupies a read port so 4x is out of reach), 6
            # tiles on DVE + 2 on ACT (Copy with per-partition scale) to
            # balance the two engines.  GpSimd compute is off-limits: its
            # ops are ~20x slower AND contend for the SBUF port shared
            # with DVE, stretching every DVE op 2-3x.  The DMA compute
            # engine only supports add-class accum ops, not mult.
            r_, sinv_, gi_ = st
            stage = opool.tile([P, T_GROUP * N_CLUST], BF16)
            for t in range(T_GROUP):
                ksl = slice(t * N_CLUST, (t + 1) * N_CLUST)
                if t < 6:
                    nc.vector.tensor_scalar_mul(
                        out=stage[:, ksl], in0=r_[:, ksl],
                        scalar1=sinv_[:, t : t + 1],
                    )
                else:
                    nc.scalar.activation(
                        out=stage[:, ksl], in_=r_[:, ksl],
                        func=mybir.ActivationFunctionType.Copy,
                        bias=0.0, scale=sinv_[:, t : t + 1],
                    )
            nc.gpsimd.dma_start(out=qv[gi_], in_=stage)

        prev = None
        for sb in range(G // SUPER):
            s0 = sb * SW
            xs = xpool.tile([P, 2 * SW], FP8, tag="xs")
            if sb == 0:
                # halve the first loads so group 0's matmuls start sooner
                for hh in range(2):
                    hsl = slice(hh * SW, (hh + 1) * SW)
                    nc.sync.dma_start(out=xs[:, hsl], in_=xdr[:, 2 * s0 + hh * SW : 2 * s0 + (hh + 1) * SW])
                # bulk affine tables land after the first x tiles so the
                # first cross matmuls are not stuck behind 1.5 MB of DMA
                nc.sync.dma_start(out=pat_s[:, N_CLUST:], in_=pat[:, N_CLUST:])
                nc.sync.dma_start(out=art_s[:, P : 2 * P], in_=art[:, P : 2 * P])
                nc.sync.dma_start(out=art_s[:, 2 * P :], in_=art[:, 2 * P :])
            else:
                nc.sync.dma_start(out=xs, in_=xdr[:, 2 * s0 : 2 * (s0 + SW)])

            for gl in range(SUPER):
                gi = sb * SUPER + gl
                ps = pspool.tile([P, T_GROUP * N_CLUST], F32)
                for t in range(T_GROUP):
                    tsl = slice(t * N_CLUST, (t + 1) * N_CLUST)
                    xcol = (gl * T_GROUP + t) * 2 * P
                    nc.tensor.matmul(
                        ps[:, tsl],
                        lhsT=xs[:, xcol : xcol + 2 * P].rearrange(
                            "p (i m) -> p i m", i=2
                        ),
                        rhs=cw8_dr,
                        start=True, stop=False,
                        perf_mode=mybir.MatmulPerfMode.DoubleRow,
                    )
                    # affine part: ps[p, (t,k)] += 256*(A_n + csq_k): K=128
                    # (zero-padded from 18) bf16 matmul on the exact same
                    # PSUM region (accumulation groups must match the open
                    # region, so this cannot batch across tiles or reorder
                    # after the next cross).
                    nc.tensor.matmul(
                        ps[:, tsl],
                        lhsT=art_s[:, gi * P : (gi + 1) * P],
                        rhs=pat_s[:, tsl],
                        start=False, stop=True,
                    )
                # r = 256 / (psum/256) = const * 1/u: one wide ACT op per
                # group straight from PSUM.  The extra /256 keeps the LUT
                # input in the ~[0.8, 4.3] domain where Reciprocal is
                # accurate; the constant factor on r cancels in the row
                # normalization.
                r = rpool.tile([P, T_GROUP * N_CLUST], BF16)
                inst = nc.scalar.activation(
                    out=r, in_=ps, bias=0.0, scale=1.0 / (XSCALE * 256.0),
                    func=mybir.ActivationFunctionType.Copy,
                )
                inst.ins.func = mybir.ActivationFunctionType.Reciprocal

                # Software pipeline: emit the PREVIOUS group's q-muls now,
                # after this group's recip.  Otherwise the ACT-side muls of
                # group g sit in the strict ACT FIFO between recip(g) and
                # recip(g+1) while waiting on the DVE sums chain, and the
                # two engines take turns idling.
                if prev is not None:
                    _emit_muls(prev)

                # Row sums: halving-add chain at 2x then a short 1x
                # tensor_reduce (tensor_reduce has no 16-bit uop; the chain
                # minimizes its 1x element count).
                r3 = r.rearrange("p (t k) -> p t k", t=T_GROUP)
                hw = 128
                hsrc = r3
                while hw >= 32:
                    ht = spool.tile([P, T_GROUP * hw], BF16, tag=f"h{hw}")
                    ht3 = ht.rearrange("p (t k) -> p t k", t=T_GROUP)
                    nc.vector.tensor_tensor(
                        out=ht3, in0=hsrc[:, :, 0:hw], in1=hsrc[:, :, hw : 2 * hw],
                        op=mybir.AluOpType.add,
                    )
                    hsrc = ht3
                    hw //= 2
                sums = spool.tile([P, T_GROUP], F32, tag="sums")
                nc.vector.tensor_reduce(
                    out=sums, in_=hsrc,
                    axis=mybir.AxisListType.X,
                    op=mybir.AluOpType.add,
                )
                # sinv on ACT (patched Reciprocal; domain sums~128 is well
                # inside the LUT's accurate range).  Emitted after the
                # previous group's ACT copies, so the only FIFO exposure is
                # recip(g+1) briefly waiting on this group's sums.
                sinv = spool.tile([P, T_GROUP], F32, tag="sinv")
                inst = nc.scalar.activation(
                    out=sinv, in_=sums, bias=0.0, scale=1.0,
                    func=mybir.ActivationFunctionType.Copy,
                )
                inst.ins.func = mybir.ActivationFunctionType.Reciprocal
                prev = (r, sinv, gi)
        _emit_muls(prev)
    nc.finalize()
    return nc


_NC_CACHE = None


def _get_nc():
    global _NC_CACHE
    if _NC_CACHE is None:
        _NC_CACHE = _build_nc()
    return _NC_CACHE


def _hi_lo_bf16(v: np.ndarray) -> tuple[np.ndarray, np.ndarray]:
    hi = v.astype(NP_BF16)
    lo = (v - hi.astype(np.float32)).astype(NP_BF16)
    return hi, lo


def kernel(x: np.ndarray, centroids: np.ndarray) -> np.ndarray:
    global LAST_RESULT
    x = np.ascontiguousarray(np.asarray(x, dtype=np.float32))
    c = np.ascontiguousarray(np.asarray(centroids, dtype=np.float32))
    assert x.shape == (N_SAMPLES, N_FEAT) and c.shape == (N_CLUST, N_FEAT)

    # Shared (replicated) centroid-side operands.
    cw8_flat = (-2.0 * XS_C * c.T).astype(NP_FP8)  # [F, K] fp8
    cw8_host = np.ascontiguousarray(
        cw8_flat.reshape(2, P, N_CLUST).transpose(1, 0, 2).reshape(P, 2 * N_CLUST)
    )
    c_sq = np.einsum("kf,kf->k", c.astype(np.float64), c.astype(np.float64))
    c_sq = (XSCALE * c_sq).astype(np.float32)
    csq_hi, csq_lo = _hi_lo_bf16(c_sq)
    pat_host = np.zeros((P, T_GROUP * N_CLUST), dtype=NP_BF16)
    for t in range(T_GROUP):
        ksl = slice(t * N_CLUST, (t + 1) * N_CLUST)
        pat_host[t, ksl] = 1.0
        pat_host[8 + t, ksl] = 1.0
        pat_host[16, ksl] = csq_hi
        pat_host[17, ksl] = csq_lo

    # m-th column consumed by the kernel (tile-major) is sample n = perm[m],
    # chosen so output partition lines are contiguous 4 KiB bursts in original
    # sample order.
    perm = np.arange(S).reshape(G, P, T_GROUP).transpose(0, 2, 1).reshape(-1)

    in_maps = []
    for i in range(N_CORES):
        xs = x[i * S : (i + 1) * S]  # [S, F]
        x_sq = np.einsum("nf,nf->n", xs.astype(np.float64), xs.astype(np.float64))
        a = (XSCALE * (1.0 + x_sq)).astype(np.float32)  # [S] = 256*A_n
        xs8 = (xs[perm] * XS_X).astype(NP_FP8)  # [S, F] fp8
        # DoubleRow pack: [T, m, i, f_lo] -> [f_lo, T, i, m]
        xdr_host = np.ascontiguousarray(
            xs8.reshape(G * T_GROUP, P, 2, P).transpose(3, 0, 2, 1).reshape(P, 2 * S)
        )
        a_hi, a_lo = _hi_lo_bf16(a)
        # art[t', g*128+p] = a_{hi,lo}[(g*128+p)*8 + t']; rows 16/17 = 1.
        art_host = np.zeros((P, G * P), dtype=NP_BF16)
        art_host[0:8] = a_hi.reshape(G * P, T_GROUP).T
        art_host[8:16] = a_lo.reshape(G * P, T_GROUP).T
        art_host[16:18] = 1.0
        in_maps.append(
            {"xdr": xdr_host, "art": np.ascontiguousarray(art_host),
             "cw8": cw8_host, "pat": pat_host}
        )

    nc = _get_nc()
    res = run_bass_kernel_spmd(
        nc, in_maps, list(range(N_CORES)), trace=RUN_TRACE
    )
    LAST_RESULT = res

    out = np.empty((N_SAMPLES, N_CLUST), dtype=np.float32)
    for i in range(N_CORES):
        out[i * S : (i + 1) * S] = res.results[i]["q"].astype(np.float32)
    return out
